# revision 7
# baseline (speedup 1.0000x reference)
"""Trainium2 Bass kernel for nn_Encoder_Layer_F (unfold -> grouped 4x4/s2 conv
-> BatchNorm(train) -> LeakyReLU(0.2) -> fold).

Sharding: the 64 locally-connected groups (8x8 patch grid) are split by patch
ROW across the 8 cores (core i owns patch row hp=i). Groups are fully
independent and BN channels belong to exactly one group, so there are no
collectives at all: each core computes its 8 groups x 256 channels over the
full batch, including exact batch statistics.

v3 (trace-driven, from 77.7us baseline -> 68.5us v2):
  * bf16 IO incl. the OUTPUT (host casts back to f32): 14 MiB/core HBM.
  * whole-group DMAs (4-8 KB per-partition contiguous runs; partition dim
    outermost in every DRAM layout), all 8 groups resident in SBUF.
  * PE pre-warm: ~26 dummy matmuls (an accumulation chain over zeros whose
    result feeds the BN epsilon constant, so DCE can't drop it) run during
    the DMA priming window -> HAM is at K=8/8 before the first real matmul.
  * priming granularity: x g0 ships in 2 parity halves, w g0 in 4 kh chunks,
    so the first matmul gates on ~512 KB instead of 1.8 MB.
  * rings: sync=w (+ last output half), scalar=x g0-3 + zh0 outputs,
    gpsimd=gb + x g4-7 + zh1 outputs; outputs split per z-half so the
    kernel tail is one 128 KB HWDGE write.
  * BN chain shortened: mean*gamma runs on DVE concurrently with the ACT
    sqrt; Prelu(scale=inv, bias=shift) applies BN+LeakyReLU in one op.

Per-core program (SPMD, identical on all cores):
  x  [128c, 8wp, 2pr, 2pc, 4qr, 4qc, 32b] bf16  parity-quadrant input
  w  [128c, 8wp, 4kh', 4kw, 256z]  bf16|f8e3  kh' = host-permuted (1,2,0,3)
  gb [128zp, 2(gamma/beta), 8wp, 2zh] f32
  o  [128zp, 8wp, 2zh, 512(oh,ow,b)]  bf16

MM_DTYPE: "bf16" = all-bf16 (rel err ~2.7e-3); "wf8" = weights in
float8_e3m4 scaled x16 on host (exactly absorbed by the batch-norm), x bf16
(rel err ~1.2e-2, halves weight DMA).
"""

import numpy as np

import concourse.bass as bass
import concourse.tile as tile
from concourse import bacc, mybir
from concourse.bass_utils import run_bass_kernel_spmd

B = 32
NC = 128
NZ = 256
HP = WP = 8
OK = 4
BN_EPS = 1e-5
LRELU = 0.2
W_F8_SCALE = 16.0
N_WARM_MM = 26

MM_DTYPE = "bf16"   # "bf16" | "wf8"
PAD_MODE = "split"  # kept for test.py compat; only split mode exists

# Real kh for each device-side kh' index: chunks arrive / are consumed in this
# order, so the first weight bytes feed the first matmuls.
KH_ORDER = [1, 2, 0, 3]


# Per-tap valid output range (stride 2, pad 1, kernel 4 on an 8-wide axis):
# i_in = 2*o + k - 1 must lie in [0, 8). k=0 -> o in [1,3]; k=3 -> o in [0,2].
def _tap_range(k):
    lo = 1 if k == 0 else 0
    hi = 2 if k == 3 else 3
    return lo, hi - lo + 1


def _taps():
    # (kp = device kh index, kh = real kh, kw). First tap (kh=1,kw=1) covers
    # the full (oh, ow) range so start=True initializes the whole PSUM tile.
    taps = []
    for kp, kh in enumerate(KH_ORDER):
        for kw in [1, 0, 2, 3] if kp == 0 else range(4):
            taps.append((kp, kh, kw))
    assert taps[0][1:] == (1, 1)
    return taps


def build_nc(mm_dtype: str = MM_DTYPE, pad_mode: str = PAD_MODE):
    f32 = mybir.dt.float32
    bf16 = mybir.dt.bfloat16
    w_dt = mybir.dt.float8e3 if mm_dtype == "wf8" else bf16

    nc = bacc.Bacc(None, target_bir_lowering=False)

    x = nc.declare_dram_parameter("x", [NC, WP, 2, 2, OK, OK, B], bf16, isOutput=False)
    w = nc.declare_dram_parameter("w", [NC, WP, 4, 4, NZ], w_dt, isOutput=False)
    gb = nc.declare_dram_parameter("gb", [128, 2, WP, 2], f32, isOutput=False)
    o = nc.declare_dram_parameter("o", [128, WP, 2, B * OK * OK], bf16, isOutput=True)

    taps = _taps()
    with tile.TileContext(nc) as tc:
        with (
            tc.tile_pool(name="xpool", bufs=WP) as xpool,
            tc.tile_pool(name="wpool", bufs=WP) as wpool,
            tc.tile_pool(name="psum", bufs=7, space=bass.MemorySpace.PSUM) as psum,
            tc.tile_pool(name="psumd", bufs=1, space=bass.MemorySpace.PSUM) as psumd,
            tc.tile_pool(name="opool", bufs=5) as opool,
            tc.tile_pool(name="spool", bufs=8) as spool,
            tc.tile_pool(name="cpool", bufs=1) as cpool,
        ):
            # ---- PE warm-up: one long accumulation chain of zero matmuls.
            # Runs while the first DMAs are in flight so the HAM clock gate
            # opens (K=8/8) before the first real matmul. The chain's result
            # (exact 0) feeds the BN epsilon constant so it can't be DCE'd.
            wd = cpool.tile([128, 128], bf16)
            nc.vector.memset(wd[:], 0.0)
            xd = cpool.tile([128, 64], bf16)
            nc.vector.memset(xd[:], 0.0)
            ptd = psumd.tile([128, 64], f32)
            for i in range(N_WARM_MM):
                nc.tensor.matmul(ptd[:], wd[:], xd[:],
                                 start=(i == 0), stop=(i == N_WARM_MM - 1))
            epst = cpool.tile([128, 1], f32)
            nc.vector.tensor_scalar_add(epst[:], ptd[:, 0:1], BN_EPS)

            gbt = cpool.tile([128, 2, WP, 2], f32)
            nc.gpsimd.dma_start(gbt[:], gb[:])

            # ---- all input DMAs up front.
            #   sync   : w g0 in 4 kh chunks, then w g1..g7 whole-group
            #   scalar : x g0 in 2 parity halves, then x g1..g3
            #   gpsimd : gb, then x g4..g7
            xts, wts = [], []
            for wp in range(WP):
                xt = xpool.tile([NC, 2, 2, OK, OK, B], bf16)
                if wp == 0:
                    nc.scalar.dma_start(xt[:, 0], x[:, 0, 0])
                    nc.scalar.dma_start(xt[:, 1], x[:, 0, 1])
                elif wp < 4:
                    nc.scalar.dma_start(xt[:], x[:, wp])
                else:
                    nc.gpsimd.dma_start(xt[:], x[:, wp])
                xts.append(xt)
                wt = wpool.tile([NC, 4, 4, NZ], w_dt)
                if wp == 0:
                    for kp in range(4):
                        nc.sync.dma_start(wt[:, kp], w[:, wp, kp])
                else:
                    nc.sync.dma_start(wt[:], w[:, wp])
                wts.append(wt)

            for wp in range(WP):
                xt, wt = xts[wp], wts[wp]
                pt0 = psum.tile([128, OK, OK, B], f32, tag="pt")
                pt1 = psum.tile([128, OK, OK, B], f32, tag="pt")
                pts = [pt0, pt1]

                def mm(zh, tap_idx):
                    kp, kh, kw = taps[tap_idx]
                    ol, oc = _tap_range(kh)
                    wl, wc = _tap_range(kw)
                    pr = (kh + 1) % 2
                    qr0 = ol + (-1 if kh == 0 else (1 if kh == 3 else 0))
                    pc = (kw + 1) % 2
                    qc0 = wl + (-1 if kw == 0 else (1 if kw == 3 else 0))
                    nc.tensor.matmul(
                        pts[zh][:, ol:ol + oc, wl:wl + wc, :],
                        wt[:, kp, kw, zh * 128:(zh + 1) * 128],
                        xt[:, pr, pc, qr0:qr0 + oc, qc0:qc0 + wc, :],
                        start=(tap_idx == 0),
                        stop=(tap_idx == len(taps) - 1),
                    )

                if wp == 0:
                    # interleave z-halves per kh chunk: each 256 KB weight
                    # chunk is fully consumed before the next is needed.
                    for kp in range(4):
                        for zh in range(2):
                            for i in range(4):
                                mm(zh, 4 * kp + i)
                else:
                    for zh in range(2):
                        for i in range(len(taps)):
                            mm(zh, i)

                ot = opool.tile([128, 2, B * OK * OK], bf16)
                for zh in range(2):
                    ptf = pts[zh].rearrange("p i j b -> p (i j b)")
                    st = spool.tile([128, 6], f32)
                    nc.vector.bn_stats(st[:], ptf)
                    mv = spool.tile([128, 2], f32)
                    nc.vector.bn_aggr(mv[:], st[:])
                    # sd = sqrt(var + eps) on ACT; mean*gamma on DVE runs
                    # concurrently.
                    sd = spool.tile([128, 1], f32)
                    nc.scalar.activation(
                        sd[:], mv[:, 1:2], mybir.ActivationFunctionType.Sqrt,
                        bias=epst[:],
                    )
                    mg = spool.tile([128, 1], f32)
                    nc.vector.tensor_mul(mg[:], mv[:, 0:1], gbt[:, 0:1, wp, zh])
                    rc = spool.tile([128, 1], f32)
                    nc.vector.reciprocal(rc[:], sd[:])
                    # inv = gamma/sd; shift = beta - mean*gamma/sd
                    inv = spool.tile([128, 1], f32)
                    nc.vector.tensor_mul(inv[:], rc[:], gbt[:, 0:1, wp, zh])
                    mgr = spool.tile([128, 1], f32)
                    nc.vector.tensor_mul(mgr[:], mg[:], rc[:])
                    sh = spool.tile([128, 1], f32)
                    nc.vector.tensor_sub(sh[:], gbt[:, 1:2, wp, zh], mgr[:])

                    # Prelu(v, alpha) == LeakyReLU(alpha) on TRN2; writes the
                    # bf16 output slice directly.
                    nc.scalar.activation(
                        ot[:, zh], ptf, mybir.ActivationFunctionType.Prelu,
                        bias=sh[:], scale=inv[:], alpha=LRELU,
                    )
                    if zh == 0:
                        nc.scalar.dma_start(o[:, wp, 0], ot[:, 0])
                    elif wp == WP - 1:
                        nc.sync.dma_start(o[:, wp, 1], ot[:, 1])
                    else:
                        nc.gpsimd.dma_start(o[:, wp, 1], ot[:, 1])

    nc.compile()
    return nc


def shard_inputs(input, weight, gamma, beta):
    """Build the 8 per-core input maps (host-side layout transforms only)."""
    import ml_dtypes
    input = np.asarray(input, dtype=np.float32)
    weight = np.asarray(weight, dtype=np.float32)
    gamma = np.asarray(gamma, dtype=np.float32)
    beta = np.asarray(beta, dtype=np.float32)

    # [B, NC, HP, 4qr, 2pr, WP, 4qc, 2pc] -> [HP, NC, WP, pr, pc, qr, qc, B]
    xs = input.reshape(B, NC, HP, OK, 2, WP, OK, 2).transpose(2, 1, 5, 4, 7, 3, 6, 0)
    xs = np.ascontiguousarray(xs, dtype=ml_dtypes.bfloat16)
    # [HP, WP, NZ, NC, 4, 4] -> [HP, NC, WP, kh, kw, NZ], kh permuted to
    # consumption order KH_ORDER.
    ws = weight.reshape(HP, WP, NZ, NC, 4, 4).transpose(0, 3, 1, 4, 5, 2)
    ws = ws[:, :, :, KH_ORDER]
    if MM_DTYPE == "wf8":
        # scale into e3m4's normal range; BN batch stats absorb it exactly.
        ws = np.ascontiguousarray(ws * W_F8_SCALE, dtype=ml_dtypes.float8_e3m4)
    else:
        ws = np.ascontiguousarray(ws, dtype=ml_dtypes.bfloat16)
    # [HP, WP, 2, 128] each -> [HP, 128zp, 2(g/b), WP, 2zh]
    gs = gamma.reshape(HP, WP, 2, 128)
    bs = beta.reshape(HP, WP, 2, 128)
    gbs = np.ascontiguousarray(
        np.stack([gs, bs], axis=1).transpose(0, 4, 1, 2, 3), dtype=np.float32)

    return [
        {"x": xs[i], "w": ws[i], "gb": gbs[i]}
        for i in range(HP)
    ]


def unshard_output(results):
    # per-core o: [128zp, WP, 2zh, (oh ow b)] -> full [B, NZ, 32, 32]
    O = np.stack([np.asarray(results[i]["o"], dtype=np.float32) for i in range(HP)])
    O = O.reshape(HP, 128, WP, 2, OK, OK, B)
    # -> [B, zh, zp, HP, oh, WP, ow]
    O = O.transpose(6, 3, 1, 0, 4, 2, 5).reshape(B, NZ, HP * OK, WP * OK)
    return np.ascontiguousarray(O)


_NC_CACHE = {}


def kernel(input, weight, gamma, beta):
    key = (MM_DTYPE, PAD_MODE)
    if key not in _NC_CACHE:
        _NC_CACHE[key] = build_nc(MM_DTYPE, PAD_MODE)
    nc = _NC_CACHE[key]
    in_maps = shard_inputs(input, weight, gamma, beta)
    res = run_bass_kernel_spmd(nc, in_maps, list(range(8))).results
    return unshard_output(res)


# revision 11
# speedup vs baseline: 1.1152x; 1.1152x over previous
"""Trainium2 Bass kernel for nn_Encoder_Layer_F (unfold -> grouped 4x4/s2 conv
-> BatchNorm(train) -> LeakyReLU(0.2) -> fold).

Sharding: the 64 locally-connected groups (8x8 patch grid) are split by patch
ROW across the 8 cores (core i owns patch row hp=i). Groups are fully
independent and BN channels belong to exactly one group, so there are no
collectives at all: each core computes its 8 groups x 256 channels over the
full batch, including exact batch statistics.

v3 (trace-driven, from 77.7us baseline -> 68.5us v2):
  * bf16 IO incl. the OUTPUT (host casts back to f32): 14 MiB/core HBM.
  * whole-group DMAs (4-8 KB per-partition contiguous runs; partition dim
    outermost in every DRAM layout), all 8 groups resident in SBUF.
  * PE pre-warm: ~26 dummy matmuls (an accumulation chain over zeros whose
    result feeds the BN epsilon constant, so DCE can't drop it) run during
    the DMA priming window -> HAM is at K=8/8 before the first real matmul.
  * priming granularity: x g0 ships in 2 parity halves, w g0 in 4 kh chunks,
    so the first matmul gates on ~512 KB instead of 1.8 MB.
  * rings: sync=w (+ last output half), scalar=x g0-3 + zh0 outputs,
    gpsimd=gb + x g4-7 + zh1 outputs; outputs split per z-half so the
    kernel tail is one 128 KB HWDGE write.
  * BN chain shortened: mean*gamma runs on DVE concurrently with the ACT
    sqrt; Prelu(scale=inv, bias=shift) applies BN+LeakyReLU in one op.

Per-core program (SPMD, identical on all cores):
  x  [128c, 8wp, 2pr, 2pc, 4qr, 4qc, 32b] bf16  parity-quadrant input
  w  [128c, 8wp, 4kh', 4kw, 256z]  bf16|f8e3  kh' = host-permuted (1,2,0,3)
  gb [128zp, 2(gamma/beta), 8wp, 2zh] f32
  o  [128zp, 8wp, 2zh, 512(oh,ow,b)]  bf16

MM_DTYPE: "bf16" = all-bf16 (rel err ~2.7e-3); "wf8" = weights in
float8_e3m4 scaled x16 on host (exactly absorbed by the batch-norm), x bf16
(rel err ~1.2e-2, halves weight DMA).
"""

import numpy as np

import concourse.bass as bass
import concourse.tile as tile
from concourse import bacc, mybir
from concourse.bass_utils import run_bass_kernel_spmd

B = 32
NC = 128
NZ = 256
HP = WP = 8
OK = 4
BN_EPS = 1e-5
LRELU = 0.2
W_F8_SCALE = 16.0
N_WARM_MM = 8

MM_DTYPE = "bf16"   # "bf16" | "wf8"
PAD_MODE = "split"  # kept for test.py compat; only split mode exists

# Real kh for each device-side kh' index: chunks arrive / are consumed in this
# order, so the first weight bytes feed the first matmuls.
KH_ORDER = [1, 2, 0, 3]


# Per-tap valid output range (stride 2, pad 1, kernel 4 on an 8-wide axis):
# i_in = 2*o + k - 1 must lie in [0, 8). k=0 -> o in [1,3]; k=3 -> o in [0,2].
def _tap_range(k):
    lo = 1 if k == 0 else 0
    hi = 2 if k == 3 else 3
    return lo, hi - lo + 1


def _taps():
    # (kp = device kh index, kh = real kh, kw). First tap (kh=1,kw=1) covers
    # the full (oh, ow) range so start=True initializes the whole PSUM tile.
    taps = []
    for kp, kh in enumerate(KH_ORDER):
        for kw in [1, 0, 2, 3] if kp == 0 else range(4):
            taps.append((kp, kh, kw))
    assert taps[0][1:] == (1, 1)
    return taps


def build_nc(mm_dtype: str = MM_DTYPE, pad_mode: str = PAD_MODE):
    f32 = mybir.dt.float32
    bf16 = mybir.dt.bfloat16
    w_dt = mybir.dt.float8e3 if mm_dtype == "wf8" else bf16

    nc = bacc.Bacc(None, target_bir_lowering=False)

    x = nc.declare_dram_parameter("x", [NC, WP, 2, 2, OK, OK, B], bf16, isOutput=False)
    w = nc.declare_dram_parameter("w", [NC, WP, 4, 4, NZ], w_dt, isOutput=False)
    gb = nc.declare_dram_parameter("gb", [128, 2, WP, 2], f32, isOutput=False)
    o = nc.declare_dram_parameter("o", [128, WP, 2, B * OK * OK], bf16, isOutput=True)

    taps = _taps()
    with tile.TileContext(nc) as tc:
        with (
            tc.tile_pool(name="xpool", bufs=WP) as xpool,
            tc.tile_pool(name="wpool", bufs=WP) as wpool,
            tc.tile_pool(name="psum", bufs=7, space=bass.MemorySpace.PSUM) as psum,
            tc.tile_pool(name="psumd", bufs=1, space=bass.MemorySpace.PSUM) as psumd,
            tc.tile_pool(name="opool", bufs=5) as opool,
            tc.tile_pool(name="spool", bufs=8) as spool,
            tc.tile_pool(name="cpool", bufs=1) as cpool,
        ):
            # ---- PE warm-up: one long accumulation chain of zero matmuls.
            # Runs while the first DMAs are in flight so the HAM clock gate
            # opens (K=8/8) before the first real matmul. The chain's result
            # (exact 0) feeds the BN epsilon constant so it can't be DCE'd.
            # ---- dummy-operand memsets + priming DMAs ride the gpsimd
            # engine, which boots ~1.3us before the HWDGE issuing engines
            # (and ~1.5us before vector runs anything): the PE warm-up chain
            # starts at ~6.2us and the first matmul's inputs land ~2us
            # earlier than via sync/scalar.
            wd = cpool.tile([128, 128], bf16)
            nc.gpsimd.memset(wd[:], 0.0)
            xd = cpool.tile([128, 512], bf16)
            nc.gpsimd.memset(xd[:], 0.0)
            xt0 = xpool.tile([NC, 2, 2, OK, OK, B], bf16, tag="xt")
            wt0 = wpool.tile([NC, 4, 4, NZ], w_dt, tag="wt")
            nc.gpsimd.dma_start(xt0[:, 0], x[:, 0, 0])
            nc.gpsimd.dma_start(wt0[:, 0], w[:, 0, 0])

            ptd = psumd.tile([128, 512], f32)
            for i in range(N_WARM_MM):
                nc.tensor.matmul(ptd[:], wd[:], xd[:],
                                 start=(i == 0), stop=(i == N_WARM_MM - 1))
            epst = cpool.tile([128, 1], f32)
            nc.vector.tensor_scalar_add(epst[:], ptd[:, 0:1], BN_EPS)

            gbt = cpool.tile([128, 2, WP, 2], f32)
            nc.gpsimd.dma_start(gbt[:], gb[:])

            # ---- remaining input DMAs up front; the two HWDGE rings carry
            # inputs EXCLUSIVELY (v3 showed output writes sharing them starve
            # the PE mid-kernel):
            #   sync   : w g0 kh chunks 1-3, then w g1..g7 whole-group
            #   scalar : x g0 parity half 1, then x g1..g7
            xts, wts = [xt0], [wt0]
            nc.sync.dma_start(wt0[:, 1], w[:, 0, 1])
            nc.scalar.dma_start(xt0[:, 1], x[:, 0, 1])
            nc.sync.dma_start(wt0[:, 2], w[:, 0, 2])
            nc.sync.dma_start(wt0[:, 3], w[:, 0, 3])
            for wp in range(1, WP):
                xt = xpool.tile([NC, 2, 2, OK, OK, B], bf16, tag="xt")
                nc.scalar.dma_start(xt[:], x[:, wp])
                xts.append(xt)
                wt = wpool.tile([NC, 4, 4, NZ], w_dt, tag="wt")
                nc.sync.dma_start(wt[:], w[:, wp])
                wts.append(wt)

            for wp in range(WP):
                xt, wt = xts[wp], wts[wp]
                pt0 = psum.tile([128, OK, OK, B], f32, tag="pt")
                pt1 = psum.tile([128, OK, OK, B], f32, tag="pt")
                pts = [pt0, pt1]

                def mm(zh, tap_idx):
                    kp, kh, kw = taps[tap_idx]
                    ol, oc = _tap_range(kh)
                    wl, wc = _tap_range(kw)
                    pr = (kh + 1) % 2
                    qr0 = ol + (-1 if kh == 0 else (1 if kh == 3 else 0))
                    pc = (kw + 1) % 2
                    qc0 = wl + (-1 if kw == 0 else (1 if kw == 3 else 0))
                    nc.tensor.matmul(
                        pts[zh][:, ol:ol + oc, wl:wl + wc, :],
                        wt[:, kp, kw, zh * 128:(zh + 1) * 128],
                        xt[:, pr, pc, qr0:qr0 + oc, qc0:qc0 + wc, :],
                        start=(tap_idx == 0),
                        stop=(tap_idx == len(taps) - 1),
                    )

                if wp == 0:
                    # interleave z-halves per kh chunk: each 256 KB weight
                    # chunk is fully consumed before the next is needed.
                    for kp in range(4):
                        for zh in range(2):
                            for i in range(4):
                                mm(zh, 4 * kp + i)
                else:
                    for zh in range(2):
                        for i in range(len(taps)):
                            mm(zh, i)

                ot = opool.tile([128, 2, B * OK * OK], bf16)
                for zh in range(2):
                    ptf = pts[zh].rearrange("p i j b -> p (i j b)")
                    st = spool.tile([128, 6], f32)
                    nc.vector.bn_stats(st[:], ptf)
                    mv = spool.tile([128, 2], f32)
                    nc.vector.bn_aggr(mv[:], st[:])
                    # sd = sqrt(var + eps) on ACT; mean*gamma on DVE runs
                    # concurrently.
                    sd = spool.tile([128, 1], f32)
                    nc.scalar.activation(
                        sd[:], mv[:, 1:2], mybir.ActivationFunctionType.Sqrt,
                        bias=epst[:],
                    )
                    mg = spool.tile([128, 1], f32)
                    nc.vector.tensor_mul(mg[:], mv[:, 0:1], gbt[:, 0:1, wp, zh])
                    rc = spool.tile([128, 1], f32)
                    nc.vector.reciprocal(rc[:], sd[:])
                    # inv = gamma/sd; shift = beta - mean*gamma/sd
                    inv = spool.tile([128, 1], f32)
                    nc.vector.tensor_mul(inv[:], rc[:], gbt[:, 0:1, wp, zh])
                    mgr = spool.tile([128, 1], f32)
                    nc.vector.tensor_mul(mgr[:], mg[:], rc[:])
                    sh = spool.tile([128, 1], f32)
                    nc.vector.tensor_sub(sh[:], gbt[:, 1:2, wp, zh], mgr[:])

                    # Prelu(v, alpha) == LeakyReLU(alpha) on TRN2; writes the
                    # bf16 output slice directly.
                    nc.scalar.activation(
                        ot[:, zh], ptf, mybir.ActivationFunctionType.Prelu,
                        bias=sh[:], scale=inv[:], alpha=LRELU,
                    )
                    # outputs ride SWDGE so the input rings stay exclusive;
                    # the very last write uses the (idle by then) sync ring
                    # to shorten the kernel tail.
                    if wp == WP - 1 and zh == 1:
                        nc.sync.dma_start(o[:, wp, 1], ot[:, 1])
                    else:
                        nc.gpsimd.dma_start(o[:, wp, zh], ot[:, zh])

    nc.compile()
    return nc


def shard_inputs(input, weight, gamma, beta):
    """Build the 8 per-core input maps (host-side layout transforms only)."""
    import ml_dtypes
    input = np.asarray(input, dtype=np.float32)
    weight = np.asarray(weight, dtype=np.float32)
    gamma = np.asarray(gamma, dtype=np.float32)
    beta = np.asarray(beta, dtype=np.float32)

    # [B, NC, HP, 4qr, 2pr, WP, 4qc, 2pc] -> [HP, NC, WP, pr, pc, qr, qc, B]
    xs = input.reshape(B, NC, HP, OK, 2, WP, OK, 2).transpose(2, 1, 5, 4, 7, 3, 6, 0)
    xs = np.ascontiguousarray(xs, dtype=ml_dtypes.bfloat16)
    # [HP, WP, NZ, NC, 4, 4] -> [HP, NC, WP, kh, kw, NZ], kh permuted to
    # consumption order KH_ORDER.
    ws = weight.reshape(HP, WP, NZ, NC, 4, 4).transpose(0, 3, 1, 4, 5, 2)
    ws = ws[:, :, :, KH_ORDER]
    if MM_DTYPE == "wf8":
        # scale into e3m4's normal range; BN batch stats absorb it exactly.
        ws = np.ascontiguousarray(ws * W_F8_SCALE, dtype=ml_dtypes.float8_e3m4)
    else:
        ws = np.ascontiguousarray(ws, dtype=ml_dtypes.bfloat16)
    # [HP, WP, 2, 128] each -> [HP, 128zp, 2(g/b), WP, 2zh]
    gs = gamma.reshape(HP, WP, 2, 128)
    bs = beta.reshape(HP, WP, 2, 128)
    gbs = np.ascontiguousarray(
        np.stack([gs, bs], axis=1).transpose(0, 4, 1, 2, 3), dtype=np.float32)

    return [
        {"x": xs[i], "w": ws[i], "gb": gbs[i]}
        for i in range(HP)
    ]


def unshard_output(results):
    # per-core o: [128zp, WP, 2zh, (oh ow b)] -> full [B, NZ, 32, 32]
    O = np.stack([np.asarray(results[i]["o"], dtype=np.float32) for i in range(HP)])
    O = O.reshape(HP, 128, WP, 2, OK, OK, B)
    # -> [B, zh, zp, HP, oh, WP, ow]
    O = O.transpose(6, 3, 1, 0, 4, 2, 5).reshape(B, NZ, HP * OK, WP * OK)
    return np.ascontiguousarray(O)


_NC_CACHE = {}


def kernel(input, weight, gamma, beta):
    key = (MM_DTYPE, PAD_MODE)
    if key not in _NC_CACHE:
        _NC_CACHE[key] = build_nc(MM_DTYPE, PAD_MODE)
    nc = _NC_CACHE[key]
    in_maps = shard_inputs(input, weight, gamma, beta)
    res = run_bass_kernel_spmd(nc, in_maps, list(range(8))).results
    return unshard_output(res)


# revision 13
# speedup vs baseline: 1.2488x; 1.1197x over previous
"""Trainium2 Bass kernel for nn_Encoder_Layer_F (unfold -> grouped 4x4/s2 conv
-> BatchNorm(train) -> LeakyReLU(0.2) -> fold).

Sharding: the 64 locally-connected groups (8x8 patch grid) are split by patch
ROW across the 8 cores (core i owns patch row hp=i). Groups are fully
independent and BN channels belong to exactly one group, so there are no
collectives at all: each core computes its 8 groups x 256 channels over the
full batch, including exact batch statistics.

v3 (trace-driven, from 77.7us baseline -> 68.5us v2):
  * bf16 IO incl. the OUTPUT (host casts back to f32): 14 MiB/core HBM.
  * whole-group DMAs (4-8 KB per-partition contiguous runs; partition dim
    outermost in every DRAM layout), all 8 groups resident in SBUF.
  * PE pre-warm: ~26 dummy matmuls (an accumulation chain over zeros whose
    result feeds the BN epsilon constant, so DCE can't drop it) run during
    the DMA priming window -> HAM is at K=8/8 before the first real matmul.
  * priming granularity: x g0 ships in 2 parity halves, w g0 in 4 kh chunks,
    so the first matmul gates on ~512 KB instead of 1.8 MB.
  * rings: sync=w (+ last output half), scalar=x g0-3 + zh0 outputs,
    gpsimd=gb + x g4-7 + zh1 outputs; outputs split per z-half so the
    kernel tail is one 128 KB HWDGE write.
  * BN chain shortened: mean*gamma runs on DVE concurrently with the ACT
    sqrt; Prelu(scale=inv, bias=shift) applies BN+LeakyReLU in one op.

Per-core program (SPMD, identical on all cores):
  x  [128c, 8wp, 2pr, 2pc, 4qr, 4qc, 32b] bf16  parity-quadrant input
  w  [128c, 8wp, 4kh', 4kw, 256z]  bf16|f8e3  kh' = host-permuted (1,2,0,3)
  gb [128zp, 2(gamma/beta), 8wp, 2zh] f32
  o  [128zp, 8wp, 2zh, 512(oh,ow,b)]  bf16

MM_DTYPE: "bf16" = all-bf16 (rel err ~2.7e-3); "wf8" = weights in
float8_e3m4 scaled x16 on host (exactly absorbed by the batch-norm), x bf16
(rel err ~1.2e-2, halves weight DMA).
"""

import numpy as np

import concourse.bass as bass
import concourse.tile as tile
from concourse import bacc, mybir
from concourse.bass_utils import run_bass_kernel_spmd

B = 32
NC = 128
NZ = 256
HP = WP = 8
OK = 4
BN_EPS = 1e-5
LRELU = 0.2
W_F8_SCALE = 16.0
N_WARM_MM = 8

MM_DTYPE = "bf16"   # "bf16" | "wf8"
PAD_MODE = "split"  # kept for test.py compat; only split mode exists

# Real kh for each device-side kh' index: chunks arrive / are consumed in this
# order, so the first weight bytes feed the first matmuls.
KH_ORDER = [1, 2, 0, 3]


# Per-tap valid output range (stride 2, pad 1, kernel 4 on an 8-wide axis):
# i_in = 2*o + k - 1 must lie in [0, 8). k=0 -> o in [1,3]; k=3 -> o in [0,2].
def _tap_range(k):
    lo = 1 if k == 0 else 0
    hi = 2 if k == 3 else 3
    return lo, hi - lo + 1


def _taps():
    # (kp = device kh index, kh = real kh, kw). First tap (kh=1,kw=1) covers
    # the full (oh, ow) range so start=True initializes the whole PSUM tile.
    taps = []
    for kp, kh in enumerate(KH_ORDER):
        for kw in [1, 0, 2, 3] if kp == 0 else range(4):
            taps.append((kp, kh, kw))
    assert taps[0][1:] == (1, 1)
    return taps


def build_nc(mm_dtype: str = MM_DTYPE, pad_mode: str = PAD_MODE):
    f32 = mybir.dt.float32
    bf16 = mybir.dt.bfloat16
    w_dt = mybir.dt.float8e3 if mm_dtype == "wf8" else bf16

    nc = bacc.Bacc(None, target_bir_lowering=False)

    x = nc.declare_dram_parameter("x", [NC, WP, 2, 2, OK, OK, B], bf16, isOutput=False)
    w = nc.declare_dram_parameter("w", [NC, WP, 4, 4, NZ], w_dt, isOutput=False)
    gb = nc.declare_dram_parameter("gb", [128, 2, WP, 2], f32, isOutput=False)
    o = nc.declare_dram_parameter("o", [128, WP, 2, B * OK * OK], bf16, isOutput=True)

    taps = _taps()
    with tile.TileContext(nc) as tc:
        with (
            tc.tile_pool(name="xpool", bufs=WP) as xpool,
            tc.tile_pool(name="wpool", bufs=WP) as wpool,
            tc.tile_pool(name="psum", bufs=7, space=bass.MemorySpace.PSUM) as psum,
            tc.tile_pool(name="psumd", bufs=1, space=bass.MemorySpace.PSUM) as psumd,
            tc.tile_pool(name="opool", bufs=5) as opool,
            tc.tile_pool(name="spool", bufs=8) as spool,
            tc.tile_pool(name="cpool", bufs=1) as cpool,
        ):
            # ---- PE warm-up: one long accumulation chain of zero matmuls.
            # Runs while the first DMAs are in flight so the HAM clock gate
            # opens (K=8/8) before the first real matmul. The chain's result
            # (exact 0) feeds the BN epsilon constant so it can't be DCE'd.
            # ---- PE warm-up: 8 N=512 dummy matmuls over zeros (~3.4us at
            # the cold 1.2 GHz clock) run while the priming DMAs are in
            # flight, so the HAM clock gate opens (K=8/8) right as the first
            # real matmul's inputs land — and the PE never idles in between
            # (an idle MID window would re-throttle it).
            wd = cpool.tile([128, 128], bf16)
            nc.vector.memset(wd[:], 0.0)
            xd = cpool.tile([128, 512], bf16)
            nc.vector.memset(xd[:], 0.0)
            xt0 = xpool.tile([NC, 2, 2, OK, OK, B], bf16, tag="xt")
            wt0 = wpool.tile([NC, 4, 4, NZ], w_dt, tag="wt")
            nc.scalar.dma_start(xt0[:, 0], x[:, 0, 0])
            nc.sync.dma_start(wt0[:, 0], w[:, 0, 0])

            ptd = psumd.tile([128, 512], f32)
            for i in range(N_WARM_MM):
                nc.tensor.matmul(ptd[:], wd[:], xd[:],
                                 start=(i == 0), stop=(i == N_WARM_MM - 1))
            epst = cpool.tile([128, 1], f32)
            nc.vector.tensor_scalar_add(epst[:], ptd[:, 0:1], BN_EPS)

            gbt = cpool.tile([128, 2, WP, 2], f32)
            nc.gpsimd.dma_start(gbt[:], gb[:])

            # ---- remaining input DMAs up front; the two HWDGE rings carry
            # inputs EXCLUSIVELY (v3 showed output writes sharing them starve
            # the PE mid-kernel):
            #   sync   : w g0 kh chunks 1-3, then w g1..g7 whole-group
            #   scalar : x g0 parity half 1, then x g1..g7
            xts, wts = [xt0], [wt0]
            nc.scalar.dma_start(xt0[:, 1], x[:, 0, 1])
            nc.sync.dma_start(wt0[:, 1], w[:, 0, 1])
            nc.sync.dma_start(wt0[:, 2], w[:, 0, 2])
            nc.sync.dma_start(wt0[:, 3], w[:, 0, 3])
            for wp in range(1, WP):
                xt = xpool.tile([NC, 2, 2, OK, OK, B], bf16, tag="xt")
                nc.scalar.dma_start(xt[:], x[:, wp])
                xts.append(xt)
                wt = wpool.tile([NC, 4, 4, NZ], w_dt, tag="wt")
                nc.sync.dma_start(wt[:], w[:, wp])
                wts.append(wt)

            for wp in range(WP):
                xt, wt = xts[wp], wts[wp]
                pt0 = psum.tile([128, OK, OK, B], f32, tag="pt")
                pt1 = psum.tile([128, OK, OK, B], f32, tag="pt")
                pts = [pt0, pt1]

                def mm(zh, tap_idx):
                    kp, kh, kw = taps[tap_idx]
                    ol, oc = _tap_range(kh)
                    wl, wc = _tap_range(kw)
                    pr = (kh + 1) % 2
                    qr0 = ol + (-1 if kh == 0 else (1 if kh == 3 else 0))
                    pc = (kw + 1) % 2
                    qc0 = wl + (-1 if kw == 0 else (1 if kw == 3 else 0))
                    nc.tensor.matmul(
                        pts[zh][:, ol:ol + oc, wl:wl + wc, :],
                        wt[:, kp, kw, zh * 128:(zh + 1) * 128],
                        xt[:, pr, pc, qr0:qr0 + oc, qc0:qc0 + wc, :],
                        start=(tap_idx == 0),
                        stop=(tap_idx == len(taps) - 1),
                    )

                if wp == 0:
                    # interleave z-halves per kh chunk: each 256 KB weight
                    # chunk is fully consumed before the next is needed.
                    for kp in range(4):
                        for zh in range(2):
                            for i in range(4):
                                mm(zh, 4 * kp + i)
                else:
                    for zh in range(2):
                        for i in range(len(taps)):
                            mm(zh, i)

                ot = opool.tile([128, 2, B * OK * OK], bf16)
                for zh in range(2):
                    ptf = pts[zh].rearrange("p i j b -> p (i j b)")
                    st = spool.tile([128, 6], f32)
                    nc.vector.bn_stats(st[:], ptf)
                    mv = spool.tile([128, 2], f32)
                    nc.vector.bn_aggr(mv[:], st[:])
                    # sd = sqrt(var + eps) on ACT; mean*gamma on DVE runs
                    # concurrently.
                    sd = spool.tile([128, 1], f32)
                    nc.scalar.activation(
                        sd[:], mv[:, 1:2], mybir.ActivationFunctionType.Sqrt,
                        bias=epst[:],
                    )
                    mg = spool.tile([128, 1], f32)
                    nc.vector.tensor_mul(mg[:], mv[:, 0:1], gbt[:, 0:1, wp, zh])
                    rc = spool.tile([128, 1], f32)
                    nc.vector.reciprocal(rc[:], sd[:])
                    # inv = gamma/sd; shift = beta - mean*gamma/sd
                    inv = spool.tile([128, 1], f32)
                    nc.vector.tensor_mul(inv[:], rc[:], gbt[:, 0:1, wp, zh])
                    mgr = spool.tile([128, 1], f32)
                    nc.vector.tensor_mul(mgr[:], mg[:], rc[:])
                    sh = spool.tile([128, 1], f32)
                    nc.vector.tensor_sub(sh[:], gbt[:, 1:2, wp, zh], mgr[:])

                    # Prelu(v, alpha) == LeakyReLU(alpha) on TRN2; writes the
                    # bf16 output slice directly.
                    nc.scalar.activation(
                        ot[:, zh], ptf, mybir.ActivationFunctionType.Prelu,
                        bias=sh[:], scale=inv[:], alpha=LRELU,
                    )
                    # outputs ride SWDGE so the input rings stay exclusive;
                    # the very last write uses the (idle by then) sync ring
                    # to shorten the kernel tail.
                    if wp == WP - 1 and zh == 1:
                        nc.sync.dma_start(o[:, wp, 1], ot[:, 1])
                    else:
                        nc.gpsimd.dma_start(o[:, wp, zh], ot[:, zh])

    nc.compile()
    return nc


def shard_inputs(input, weight, gamma, beta):
    """Build the 8 per-core input maps (host-side layout transforms only)."""
    import ml_dtypes
    input = np.asarray(input, dtype=np.float32)
    weight = np.asarray(weight, dtype=np.float32)
    gamma = np.asarray(gamma, dtype=np.float32)
    beta = np.asarray(beta, dtype=np.float32)

    # [B, NC, HP, 4qr, 2pr, WP, 4qc, 2pc] -> [HP, NC, WP, pr, pc, qr, qc, B]
    xs = input.reshape(B, NC, HP, OK, 2, WP, OK, 2).transpose(2, 1, 5, 4, 7, 3, 6, 0)
    xs = np.ascontiguousarray(xs, dtype=ml_dtypes.bfloat16)
    # [HP, WP, NZ, NC, 4, 4] -> [HP, NC, WP, kh, kw, NZ], kh permuted to
    # consumption order KH_ORDER.
    ws = weight.reshape(HP, WP, NZ, NC, 4, 4).transpose(0, 3, 1, 4, 5, 2)
    ws = ws[:, :, :, KH_ORDER]
    if MM_DTYPE == "wf8":
        # scale into e3m4's normal range; BN batch stats absorb it exactly.
        ws = np.ascontiguousarray(ws * W_F8_SCALE, dtype=ml_dtypes.float8_e3m4)
    else:
        ws = np.ascontiguousarray(ws, dtype=ml_dtypes.bfloat16)
    # [HP, WP, 2, 128] each -> [HP, 128zp, 2(g/b), WP, 2zh]
    gs = gamma.reshape(HP, WP, 2, 128)
    bs = beta.reshape(HP, WP, 2, 128)
    gbs = np.ascontiguousarray(
        np.stack([gs, bs], axis=1).transpose(0, 4, 1, 2, 3), dtype=np.float32)

    return [
        {"x": xs[i], "w": ws[i], "gb": gbs[i]}
        for i in range(HP)
    ]


def unshard_output(results):
    # per-core o: [128zp, WP, 2zh, (oh ow b)] -> full [B, NZ, 32, 32]
    O = np.stack([np.asarray(results[i]["o"], dtype=np.float32) for i in range(HP)])
    O = O.reshape(HP, 128, WP, 2, OK, OK, B)
    # -> [B, zh, zp, HP, oh, WP, ow]
    O = O.transpose(6, 3, 1, 0, 4, 2, 5).reshape(B, NZ, HP * OK, WP * OK)
    return np.ascontiguousarray(O)


_NC_CACHE = {}


def kernel(input, weight, gamma, beta):
    key = (MM_DTYPE, PAD_MODE)
    if key not in _NC_CACHE:
        _NC_CACHE[key] = build_nc(MM_DTYPE, PAD_MODE)
    nc = _NC_CACHE[key]
    in_maps = shard_inputs(input, weight, gamma, beta)
    res = run_bass_kernel_spmd(nc, in_maps, list(range(8))).results
    return unshard_output(res)
